# revision 7
# baseline (speedup 1.0000x reference)
"""Trainium2 Bass kernel for AttentionAggregator (GNN message passing).

  x      = prev_hidden @ W1.T                       [N, D]
  scores = einsum('nkd,nd->nk', neigh_hidden, x)    [N, K]
  attn   = softmax(where(mask, -inf, scores), k)    [N, K]
  agg    = einsum('nk,nkd->nd', attn, neigh_hidden) [N, D]
  out    = tanh(concat([agg, prev_hidden], 1) @ Wa.T)

Sharding: node dim N split evenly across 8 NeuronCores (data parallel);
W1 / Wa replicated.  Per-core kernel is DMA-bound (neigh_hidden stream).

Per-tile schedule (P<=128 nodes on partitions):
  PE : prev transposes, x = prevT.T @ W1T (fp32r), agg transposes,
       out = catT.T @ WaT (fp32r)
  DVE: scores via fused tensor_tensor_reduce (mask folded into init),
       softmax reductions, weighted-sum accumulate (scalar_tensor_tensor)
  ACT: exp, attn normalize, PSUM->SBUF copies, tanh
"""

import sys

sys.path.insert(0, "/opt/trn_rl_repo")

import numpy as np
from contextlib import ExitStack

import concourse.bass as bass
import concourse.tile as tile
from concourse import bacc, mybir
from concourse.bass_utils import run_bass_kernel_spmd
from concourse.masks import make_identity

F32 = mybir.dt.float32
F32R = mybir.dt.float32r
U8 = mybir.dt.uint8
ALU = mybir.AluOpType
ACTF = mybir.ActivationFunctionType
AX = mybir.AxisListType

N, K, D = 20000, 32, 512
N_CORES = 8
NS = N // N_CORES  # nodes per core

NEG_BIG = -1.0e30


def _r(ap):
    return ap.bitcast(F32R)


def build_nc(ns=NS, agg_split=0):
    """Build the per-core Tile program.

    agg_split: number of k indices of the weighted-sum accumulation that run
    on GPSIMD (Pool) instead of DVE, to balance the two engines.
    """
    nc = bacc.Bacc("TRN2", target_bir_lowering=False, debug=False)

    prev_d = nc.dram_tensor("prev", [ns, D], F32, kind="ExternalInput").ap()
    neigh_d = nc.dram_tensor("neigh", [ns, K, D], F32, kind="ExternalInput").ap()
    mask_d = nc.dram_tensor("mask", [ns, K], U8, kind="ExternalInput").ap()
    w1_d = nc.dram_tensor("W1", [D, D], F32, kind="ExternalInput").ap()
    wa_d = nc.dram_tensor("Wa", [D, 2 * D], F32, kind="ExternalInput").ap()
    out_d = nc.dram_tensor("out", [ns, D], F32, kind="ExternalOutput").ap()

    n_tiles = (ns + 127) // 128
    DC = D // 128  # 4 d-chunks
    CC = 2 * D // 128  # 8 concat-chunks

    with tile.TileContext(nc) as tc, ExitStack() as ctx:
        consts = ctx.enter_context(tc.tile_pool(name="consts", bufs=1))
        wstage = ctx.enter_context(tc.tile_pool(name="wstage", bufs=2))
        p_neigh = ctx.enter_context(tc.tile_pool(name="neigh", bufs=2))
        p_io = ctx.enter_context(tc.tile_pool(name="io", bufs=2))
        p_big = ctx.enter_context(tc.tile_pool(name="big", bufs=2))
        p_small = ctx.enter_context(tc.tile_pool(name="small", bufs=2))
        p_ps_tr = ctx.enter_context(tc.tile_pool(name="ps_tr", bufs=2, space="PSUM"))
        p_ps_x = ctx.enter_context(tc.tile_pool(name="ps_x", bufs=2, space="PSUM"))
        p_ps_o = ctx.enter_context(tc.tile_pool(name="ps_o", bufs=2, space="PSUM"))

        # ---- one-time: identity + transposed weights ----
        ident = consts.tile([128, 128], F32)
        make_identity(nc, ident[:])

        # W1T[i, j] = W1[j, i] : DC tiles of [128 i, D j], packed [128, DC, D]
        w1t = consts.tile([128, DC, D], F32)
        for cj in range(DC):
            wrow = wstage.tile([128, D], F32, tag="wrow")
            nc.sync.dma_start(wrow[:], w1_d[cj * 128 : (cj + 1) * 128, :])
            for ci in range(DC):
                tp = p_ps_tr.tile([128, 128], F32, tag="wtp")
                nc.tensor.transpose(tp[:], wrow[:, ci * 128 : (ci + 1) * 128], ident[:])
                nc.scalar.copy(w1t[:, ci, cj * 128 : (cj + 1) * 128], tp[:])

        # WaT[c, j] = Wa[j, c] : CC tiles of [128 c, D j], packed [128, CC, D]
        wat = consts.tile([128, CC, D], F32)
        for cj in range(DC):
            wrow = wstage.tile([128, 2 * D], F32, tag="warow")
            nc.sync.dma_start(wrow[:], wa_d[cj * 128 : (cj + 1) * 128, :])
            for ci in range(CC):
                tp = p_ps_tr.tile([128, 128], F32, tag="wtp")
                nc.tensor.transpose(tp[:], wrow[:, ci * 128 : (ci + 1) * 128], ident[:])
                nc.scalar.copy(wat[:, ci, cj * 128 : (cj + 1) * 128], tp[:])

        # ---- per-tile loop ----
        for t in range(n_tiles):
            n0 = t * 128
            P = min(128, ns - n0)

            # loads
            nghs = []
            for c in range(K // 2):  # 16 DMAs of [P, 2, D] to spread across queues
                g = p_neigh.tile([P, 2, D], F32, tag=f"ng{c}")
                nc.sync.dma_start(g[:], neigh_d[n0 : n0 + P, 2 * c : 2 * c + 2, :])
                nghs.append(g)

            def ng(k):
                return nghs[k // 2][:, k % 2, :]

            prev_t = p_io.tile([P, D], F32, tag="prev")
            nc.sync.dma_start(prev_t[:], prev_d[n0 : n0 + P, :])
            mask_t = p_small.tile([P, K], U8, tag="mask")
            nc.sync.dma_start(mask_t[:], mask_d[n0 : n0 + P, :])

            # catT[c, n]: chunks 0..DC-1 = aggT, DC..CC-1 = prevT
            catT = p_big.tile([128, CC, P], F32, tag="catT")
            for ci in range(DC):
                tp = p_ps_tr.tile([128, P], F32, tag="tp")
                nc.tensor.transpose(
                    tp[:], prev_t[:, ci * 128 : (ci + 1) * 128], ident[:P, :P]
                )
                nc.scalar.copy(catT[:, DC + ci, :], tp[:])

            # x = prev @ W1.T  -> PSUM [P, D], then SBUF
            x_ps = p_ps_x.tile([P, D], F32, tag="x")
            for ci in range(DC):
                nc.tensor.matmul(
                    x_ps[:],
                    catT[:, DC + ci, :],
                    w1t[:, ci, :],
                    start=(ci == 0),
                    stop=(ci == DC - 1),
                )
            x_sb = p_io.tile([P, D], F32, tag="x_sb")
            nc.scalar.copy(x_sb[:], x_ps[:])

            # mask penalty (u8 -> f32 * -1e30)
            maskpen = p_small.tile([P, K], F32, tag="maskpen")
            nc.vector.tensor_scalar_mul(maskpen[:], mask_t[:], NEG_BIG)

            # scores_k = sum_d neigh_k * x  (fused dot via scalar_tensor_tensor
            # accum_out; InstTensorTensorReduce crashes TRN2 hw here)
            scores = p_small.tile([P, K], F32, tag="scores")
            scratch = p_io.tile([P, D], F32, tag="scratch")
            for k in range(K):
                nc.vector.scalar_tensor_tensor(
                    out=scratch[:],
                    in0=ng(k),
                    scalar=1.0,
                    in1=x_sb[:],
                    op0=ALU.bypass,
                    op1=ALU.mult,
                    accum_out=scores[:, k : k + 1],
                )
            nc.vector.tensor_tensor(
                out=scores[:], in0=scores[:], in1=maskpen[:], op=ALU.add
            )

            # softmax over k (free dim)
            nmx = p_small.tile([P, 1], F32, tag="nmx")
            nc.vector.tensor_reduce(nmx[:], scores[:], axis=AX.X, op=ALU.max, negate=True)
            ex = p_small.tile([P, K], F32, tag="ex")
            nc.scalar.activation(ex[:], scores[:], ACTF.Exp, bias=nmx[:, 0:1], scale=1.0)
            ssum = p_small.tile([P, 1], F32, tag="ssum")
            nc.vector.tensor_reduce(ssum[:], ex[:], axis=AX.X, op=ALU.add)
            rec = p_small.tile([P, 1], F32, tag="rec")
            nc.vector.reciprocal(rec[:], ssum[:])
            attn = p_small.tile([P, K], F32, tag="attn")
            nc.scalar.activation(attn[:], ex[:], ACTF.Copy, bias=0.0, scale=rec[:, 0:1])

            # agg = sum_k attn_k * neigh_k  (DVE + GPSIMD split)
            agg = p_io.tile([P, D], F32, tag="agg")
            n_pool = max(0, min(K - 2, agg_split))
            agg2 = p_io.tile([P, D], F32, tag="agg2", name="agg2") if n_pool else None
            kd = list(range(K - n_pool))  # DVE ks (accumulate into agg)
            kp = list(range(K - n_pool, K))  # GPSIMD ks (accumulate into agg2)
            nc.vector.scalar_tensor_tensor(
                out=agg[:], in0=ng(kd[0]), scalar=attn[:, kd[0] : kd[0] + 1],
                in1=x_sb[:], op0=ALU.mult, op1=ALU.bypass,
            )
            for k in kd[1:]:
                nc.vector.scalar_tensor_tensor(
                    out=agg[:], in0=ng(k), scalar=attn[:, k : k + 1],
                    in1=agg[:], op0=ALU.mult, op1=ALU.add,
                )
            if kp:
                nc.gpsimd.scalar_tensor_tensor(
                    out=agg2[:], in0=ng(kp[0]), scalar=attn[:, kp[0] : kp[0] + 1],
                    in1=x_sb[:], op0=ALU.mult, op1=ALU.bypass,
                )
                for k in kp[1:]:
                    nc.gpsimd.scalar_tensor_tensor(
                        out=agg2[:], in0=ng(k), scalar=attn[:, k : k + 1],
                        in1=agg2[:], op0=ALU.mult, op1=ALU.add,
                    )
                nc.vector.tensor_tensor(
                    out=agg[:], in0=agg[:], in1=agg2[:], op=ALU.add
                )

            # aggT into catT chunks 0..DC-1
            for ci in range(DC):
                tp = p_ps_tr.tile([128, P], F32, tag="tp")
                nc.tensor.transpose(
                    tp[:], agg[:, ci * 128 : (ci + 1) * 128], ident[:P, :P]
                )
                nc.scalar.copy(catT[:, ci, :], tp[:])

            # out = tanh(cat @ Wa.T)
            o_ps = p_ps_o.tile([P, D], F32, tag="o")
            for ci in range(CC):
                nc.tensor.matmul(
                    o_ps[:],
                    catT[:, ci, :],
                    wat[:, ci, :],
                    start=(ci == 0),
                    stop=(ci == CC - 1),
                )
            out_sb = p_io.tile([P, D], F32, tag="out_sb")
            nc.scalar.activation(out_sb[:], o_ps[:], ACTF.Tanh)
            nc.sync.dma_start(out_d[n0 : n0 + P, :], out_sb[:])

    nc.compile()
    return nc


_NC_CACHE = {}


def _get_nc(ns=NS, agg_split=0):
    key = (ns, agg_split)
    if key not in _NC_CACHE:
        _NC_CACHE[key] = build_nc(ns, agg_split)
    return _NC_CACHE[key]


def kernel(prev_hidden, neigh_hidden, mask, W1, Wa, _trace=False, _tmpdir=None):
    prev_hidden = np.ascontiguousarray(prev_hidden, dtype=np.float32)
    neigh_hidden = np.ascontiguousarray(neigh_hidden, dtype=np.float32)
    mask_u8 = np.ascontiguousarray(mask).view(np.uint8)
    W1 = np.ascontiguousarray(W1, dtype=np.float32)
    Wa = np.ascontiguousarray(Wa, dtype=np.float32)

    nc = _get_nc()
    in_maps = []
    for c in range(N_CORES):
        s = slice(c * NS, (c + 1) * NS)
        in_maps.append(
            {
                "prev": prev_hidden[s],
                "neigh": neigh_hidden[s],
                "mask": mask_u8[s],
                "W1": W1,
                "Wa": Wa,
            }
        )
    res = run_bass_kernel_spmd(
        nc,
        in_maps,
        core_ids=list(range(N_CORES)),
        trace=_trace,
        tmpdir=_tmpdir,
    )
    out = np.concatenate([res.results[c]["out"] for c in range(N_CORES)], axis=0)
    if _trace:
        kernel.last_exec_time_ns = res.exec_time_ns
    return out


# revision 13
# speedup vs baseline: 1.1697x; 1.1697x over previous
"""Trainium2 Bass kernel for AttentionAggregator (GNN message passing).

  x      = prev_hidden @ W1.T                       [N, D]
  scores = einsum('nkd,nd->nk', neigh_hidden, x)    [N, K]
  attn   = softmax(where(mask, -inf, scores), k)    [N, K]
  agg    = einsum('nk,nkd->nd', attn, neigh_hidden) [N, K, D] -> [N, D]
  out    = tanh(concat([agg, prev_hidden], 1) @ Wa.T)

Sharding: node dim N split evenly across 8 NeuronCores (data parallel);
W1 / Wa replicated.  Per-core kernel is DMA-bound (neigh_hidden stream,
~164MB/core) with DVE as the #2 engine.

Per-tile schedule (P<=128 nodes on partitions):
  DVE    : scores via fused dot (scalar_tensor_tensor accum_out), softmax
           reductions, reciprocal
  GPSIMD : builds diag(attn[:,k]) for all k in one affine_select
  PE     : prev/agg transposes, x = prevT.T@W1T (plain fp32 for exact
           logits), agg = sum_k diag(attn_k)@neigh_k (fp32r, PSUM accum),
           out = catT.T@WaT (fp32r)
  ACT    : exp, attn normalize, PSUM->SBUF copies, tanh
"""

import sys

sys.path.insert(0, "/opt/trn_rl_repo")

import numpy as np
from contextlib import ExitStack

import concourse.bass as bass
import concourse.tile as tile
from concourse import bacc, mybir
from concourse.bass_utils import run_bass_kernel_spmd
from concourse.masks import make_identity

F32 = mybir.dt.float32
F32R = mybir.dt.float32r
U8 = mybir.dt.uint8
ALU = mybir.AluOpType
ACTF = mybir.ActivationFunctionType
AX = mybir.AxisListType

N, K, D = 20000, 32, 512
N_CORES = 8
NS = N // N_CORES  # nodes per core

NEG_BIG = -1.0e30


def build_nc(ns=NS, agg_mode="pe", debug_taps=False):
    """Build the per-core Tile program.

    agg_mode: 'pe'  — aggregate via diag(attn_k) matmuls on the tensor engine
              'dve' — aggregate via scalar_tensor_tensor chain on DVE
    """
    nc = bacc.Bacc("TRN2", target_bir_lowering=False, debug=False)

    prev_d = nc.dram_tensor("prev", [ns, D], F32, kind="ExternalInput").ap()
    # declared f32r so tiles can feed fp32r matmuls; bit-identical to f32
    neigh_d = nc.dram_tensor("neigh", [ns, K, D], F32, kind="ExternalInput").ap()
    mask_d = nc.dram_tensor("mask", [ns, K], U8, kind="ExternalInput").ap()
    w1_d = nc.dram_tensor("W1", [D, D], F32, kind="ExternalInput").ap()
    wa_d = nc.dram_tensor("Wa", [D, 2 * D], F32, kind="ExternalInput").ap()
    out_d = nc.dram_tensor("out", [ns, D], F32, kind="ExternalOutput").ap()
    taps = {}
    if debug_taps:
        for nm, sh in [("x", [ns, D]), ("scores", [ns, K]), ("attn", [ns, K]), ("agg", [ns, D])]:
            taps[nm] = nc.dram_tensor("tap_" + nm, sh, F32, kind="ExternalOutput").ap()

    n_tiles = (ns + 127) // 128
    DC = D // 128  # 4 d-chunks
    CC = 2 * D // 128  # 8 concat-chunks
    NCH = 8  # neigh DMA chunks per tile
    KCH = K // NCH  # k's per chunk

    with tile.TileContext(nc) as tc, ExitStack() as ctx:
        consts = ctx.enter_context(tc.tile_pool(name="consts", bufs=1))
        wstage = ctx.enter_context(tc.tile_pool(name="wstage", bufs=1))
        p_neigh = ctx.enter_context(tc.tile_pool(name="neigh", bufs=2))
        p_io = ctx.enter_context(tc.tile_pool(name="io", bufs=2))
        p_scr = ctx.enter_context(tc.tile_pool(name="scr", bufs=1))
        p_big = ctx.enter_context(tc.tile_pool(name="big", bufs=2))
        p_small = ctx.enter_context(tc.tile_pool(name="small", bufs=2))
        p_diag = ctx.enter_context(tc.tile_pool(name="diag", bufs=1))
        p_stg = ctx.enter_context(tc.tile_pool(name="stg", bufs=3))
        p_ps_tr = ctx.enter_context(tc.tile_pool(name="ps_tr", bufs=2, space="PSUM"))
        p_ps_x = ctx.enter_context(tc.tile_pool(name="ps_x", bufs=2, space="PSUM"))
        p_ps_a = ctx.enter_context(tc.tile_pool(name="ps_a", bufs=2, space="PSUM"))
        p_ps_o = ctx.enter_context(tc.tile_pool(name="ps_o", bufs=2, space="PSUM"))

        # ---- one-time: identity + transposed weights ----
        ident = consts.tile([128, 128], F32)
        make_identity(nc, ident[:])

        # W1T[i, j] = W1[j, i] packed [128, DC, D] (f32: x-matmul is exact fp32)
        w1t = consts.tile([128, DC, D], F32)
        for cj in range(DC):
            wrow = wstage.tile([128, D], F32, tag="wrow")
            nc.sync.dma_start(wrow[:], w1_d[cj * 128 : (cj + 1) * 128, :])
            for ci in range(DC):
                tp = p_ps_tr.tile([128, 128], F32, tag="tp", name="tp")
                nc.tensor.transpose(tp[:], wrow[:, ci * 128 : (ci + 1) * 128], ident[:])
                nc.scalar.copy(w1t[:, ci, cj * 128 : (cj + 1) * 128], tp[:])

        # WaT[c, j] = Wa[j, c] packed [128, CC, D] (f32: out-matmul is exact fp32)
        wat = consts.tile([128, CC, D], F32)
        for cj in range(DC):
            wrow = wstage.tile([128, 2 * D], F32, tag="warow")
            nc.sync.dma_start(wrow[:], wa_d[cj * 128 : (cj + 1) * 128, :])
            for ci in range(CC):
                tp = p_ps_tr.tile([128, 128], F32, tag="tp", name="tp")
                nc.tensor.transpose(tp[:], wrow[:, ci * 128 : (ci + 1) * 128], ident[:])
                nc.scalar.copy(wat[:, ci, cj * 128 : (cj + 1) * 128], tp[:])

        # ---- per-tile loop (software pipelined: x for tile t+1 is computed
        # on PE before tile t's aggregation matmuls so DVE never starves) ----
        state = {}

        def emit_loads(t):
            n0 = t * 128
            P = min(128, ns - n0)
            nghs = []
            for c in range(NCH):
                g = p_neigh.tile([P, KCH, D], F32, tag=f"ng{c}", name=f"ng{c}")
                nc.sync.dma_start(
                    g[:], neigh_d[n0 : n0 + P, KCH * c : KCH * (c + 1), :]
                )
                nghs.append(g)
            prev_t = p_io.tile([P, D], F32, tag="prev", name="prev_t")
            nc.scalar.dma_start(prev_t[:], prev_d[n0 : n0 + P, :])
            mask_t = p_small.tile([P, K], U8, tag="mask", name="mask_t")
            nc.scalar.dma_start(mask_t[:], mask_d[n0 : n0 + P, :])
            return {"nghs": nghs, "prev": prev_t, "mask": mask_t, "P": P, "n0": n0}

        def emit_x(st):
            P = st["P"]
            catT = p_big.tile([128, CC, P], F32, tag="catT", name="catT")
            prevT = p_io.tile([128, DC, P], F32, tag="prevT", name="prevT")
            for ci in range(DC):
                tp = p_ps_tr.tile([128, P], F32, tag="tp", name="tp")
                nc.tensor.transpose(
                    tp[:], st["prev"][:, ci * 128 : (ci + 1) * 128], ident[:P, :P]
                )
                nc.scalar.copy(prevT[:, ci, :], tp[:])
                nc.scalar.copy(catT[:, DC + ci, :], tp[:])
            x_ps = p_ps_x.tile([P, D], F32, tag="x", name="x_ps")
            for ci in range(DC):
                nc.tensor.matmul(
                    x_ps[:],
                    prevT[:, ci, :],
                    w1t[:, ci, :],
                    start=(ci == 0),
                    stop=(ci == DC - 1),
                )
            x_sb = p_io.tile([P, D], F32, tag="x_sb", name="x_sb")
            nc.scalar.copy(x_sb[:], x_ps[:])
            if debug_taps:
                nc.sync.dma_start(taps["x"][st["n0"] : st["n0"] + P, :], x_sb[:])
            st["catT"] = catT
            st["x_sb"] = x_sb

        def emit_body(st):
            P, n0 = st["P"], st["n0"]
            nghs, x_sb, catT = st["nghs"], st["x_sb"], st["catT"]

            def ng(k):
                return nghs[k // KCH][:, k % KCH, :]

            # mask penalty (u8 -> f32 * -1e30)
            maskpen = p_small.tile([P, K], F32, tag="maskpen", name="maskpen")
            nc.vector.tensor_scalar_mul(maskpen[:], st["mask"][:], NEG_BIG)

            # scores_k = sum_d neigh_k * x  (fused dot via scalar_tensor_tensor
            # accum_out; InstTensorTensorReduce crashes TRN2 hw here)
            scores = p_small.tile([P, K], F32, tag="scores", name="scores")
            scratch = p_scr.tile([P, D], F32, tag="scratch", name="scratch")
            for k in range(K):
                nc.vector.scalar_tensor_tensor(
                    out=scratch[:],
                    in0=ng(k),
                    scalar=1.0,
                    in1=x_sb[:],
                    op0=ALU.bypass,
                    op1=ALU.mult,
                    accum_out=scores[:, k : k + 1],
                )
            nc.vector.tensor_tensor(
                out=scores[:], in0=scores[:], in1=maskpen[:], op=ALU.add
            )

            if debug_taps:
                nc.sync.dma_start(taps["scores"][n0 : n0 + P, :], scores[:])
            # softmax over k (free dim)
            nmx = p_small.tile([P, 1], F32, tag="nmx", name="nmx")
            nc.vector.tensor_reduce(
                nmx[:], scores[:], axis=AX.X, op=ALU.max, negate=True
            )
            ex = p_small.tile([P, K], F32, tag="ex", name="ex")
            nc.scalar.activation(ex[:], scores[:], ACTF.Exp, bias=nmx[:, 0:1], scale=1.0)
            ssum = p_small.tile([P, 1], F32, tag="ssum", name="ssum")
            nc.vector.tensor_reduce(ssum[:], ex[:], axis=AX.X, op=ALU.add)
            rec = p_small.tile([P, 1], F32, tag="rec", name="rec")
            nc.vector.reciprocal(rec[:], ssum[:])
            attn = p_small.tile([P, K], F32, tag="attn", name="attn")
            nc.scalar.activation(attn[:], ex[:], ACTF.Copy, bias=0.0, scale=rec[:, 0:1])

            if debug_taps:
                nc.sync.dma_start(taps["attn"][n0 : n0 + P, :], attn[:])
            if agg_mode == "pe":
                # f32r staging copies of neigh (producer-rounded for the fp32r
                # matmul; the F32 originals keep DVE scores exact)
                stgs = []
                for c in range(K // 2):
                    stg = p_stg.tile([P, 2, D], F32R, tag="stg", name="stg")
                    src = nghs[(2 * c) // KCH][:, (2 * c) % KCH : (2 * c) % KCH + 2, :]
                    if c % 2 == 0:
                        nc.gpsimd.tensor_copy(stg[:], src)
                    else:
                        nc.scalar.copy(stg[:], src)
                    stgs.append(stg)

                def ngr(k):
                    return stgs[k // 2][:, k % 2, :]

                # diag(attn_k) for all k in one gpsimd affine_select:
                # diags[p, k, f] = attn[p, k] if p == f else 0
                KH = K // 2
                agg_ps = p_ps_a.tile([P, D], F32, tag="agg", name="agg_ps")
                for h in range(2):
                    diags = p_diag.tile([128, KH, 128], F32R, tag="diags", name="diags")
                    attn_b = (
                        attn[:, h * KH : (h + 1) * KH]
                        .unsqueeze(2)
                        .broadcast_to([P, KH, 128])
                    )
                    nc.gpsimd.affine_select(
                        out=diags[:P, :, :128],
                        in_=attn_b,
                        pattern=[[0, KH], [-1, 128]],
                        compare_op=ALU.is_equal,
                        fill=0.0,
                        base=0,
                        channel_multiplier=1,
                    )
                    # agg += diag(attn_k) @ neigh_k (fp32r; single product per
                    # output so fp32r rounding stays ~1e-4)
                    for kk in range(KH):
                        k = h * KH + kk
                        nc.tensor.matmul(
                            agg_ps[:],
                            diags[:P, kk, :P],
                            ngr(k),
                            start=(k == 0),
                            stop=(k == K - 1),
                        )
                agg = p_scr.tile([P, D], F32, tag="agg", name="agg")
                nc.scalar.copy(agg[:], agg_ps[:])
            else:
                agg = p_io.tile([P, D], F32, tag="agg", name="agg")
                nc.vector.scalar_tensor_tensor(
                    out=agg[:], in0=ng(0), scalar=attn[:, 0:1],
                    in1=x_sb[:], op0=ALU.mult, op1=ALU.bypass,
                )
                for k in range(1, K):
                    nc.vector.scalar_tensor_tensor(
                        out=agg[:], in0=ng(k), scalar=attn[:, k : k + 1],
                        in1=agg[:], op0=ALU.mult, op1=ALU.add,
                    )

            if debug_taps:
                nc.sync.dma_start(taps["agg"][n0 : n0 + P, :], agg[:])
            # aggT into catT chunks 0..DC-1
            for ci in range(DC):
                tp = p_ps_tr.tile([128, P], F32, tag="tp", name="tp")
                nc.tensor.transpose(
                    tp[:], agg[:, ci * 128 : (ci + 1) * 128], ident[:P, :P]
                )
                nc.scalar.copy(catT[:, ci, :], tp[:])

            # out = tanh(cat @ Wa.T)   (plain fp32: 1024-long contraction is
            # too error-prone in fp32r)
            o_ps = p_ps_o.tile([P, D], F32, tag="o", name="o_ps")
            for ci in range(CC):
                nc.tensor.matmul(
                    o_ps[:],
                    catT[:, ci, :],
                    wat[:, ci, :],
                    start=(ci == 0),
                    stop=(ci == CC - 1),
                )
            out_sb = p_io.tile([P, D], F32, tag="out_sb", name="out_sb")
            nc.scalar.activation(out_sb[:], o_ps[:], ACTF.Tanh)
            nc.scalar.dma_start(out_d[n0 : n0 + P, :], out_sb[:])

        # prologue
        cur = emit_loads(0)
        emit_x(cur)
        for t in range(n_tiles):
            nxt = None
            if t + 1 < n_tiles:
                nxt = emit_loads(t + 1)
                emit_x(nxt)
            emit_body(cur)
            cur = nxt

    nc.compile()
    return nc


_NC_CACHE = {}


def _get_nc(ns=NS, agg_mode="pe"):
    key = (ns, agg_mode)
    if key not in _NC_CACHE:
        _NC_CACHE[key] = build_nc(ns, agg_mode)
    return _NC_CACHE[key]


def kernel(prev_hidden, neigh_hidden, mask, W1, Wa, _trace=False, _tmpdir=None):
    prev_hidden = np.ascontiguousarray(prev_hidden, dtype=np.float32)
    neigh_hidden = np.ascontiguousarray(neigh_hidden, dtype=np.float32)
    mask_u8 = np.ascontiguousarray(mask).view(np.uint8)
    W1 = np.ascontiguousarray(W1, dtype=np.float32)
    Wa = np.ascontiguousarray(Wa, dtype=np.float32)

    nc = _get_nc()
    in_maps = []
    for c in range(N_CORES):
        s = slice(c * NS, (c + 1) * NS)
        in_maps.append(
            {
                "prev": prev_hidden[s],
                "neigh": neigh_hidden[s],
                "mask": mask_u8[s],
                "W1": W1,
                "Wa": Wa,
            }
        )
    res = run_bass_kernel_spmd(
        nc,
        in_maps,
        core_ids=list(range(N_CORES)),
        trace=_trace,
        tmpdir=_tmpdir,
    )
    out = np.concatenate([res.results[c]["out"] for c in range(N_CORES)], axis=0)
    if _trace:
        kernel.last_exec_time_ns = res.exec_time_ns
    return out


# revision 14
# speedup vs baseline: 1.3515x; 1.1554x over previous
"""Trainium2 Bass kernel for AttentionAggregator (GNN message passing).

  x      = prev_hidden @ W1.T                       [N, D]
  scores = einsum('nkd,nd->nk', neigh_hidden, x)    [N, K]
  attn   = softmax(where(mask, -inf, scores), k)    [N, K]
  agg    = einsum('nk,nkd->nd', attn, neigh_hidden) [N, K, D] -> [N, D]
  out    = tanh(concat([agg, prev_hidden], 1) @ Wa.T)

Sharding: node dim N split evenly across 8 NeuronCores (data parallel);
W1 / Wa replicated.  Per-core kernel is DMA-bound (neigh_hidden stream,
~164MB/core) with DVE as the #2 engine.

Per-tile schedule (P<=128 nodes on partitions):
  DVE    : scores via fused dot (scalar_tensor_tensor accum_out), softmax
           reductions, reciprocal
  GPSIMD : builds diag(attn[:,k]) for all k in one affine_select
  PE     : prev/agg transposes, x = prevT.T@W1T (plain fp32 for exact
           logits), agg = sum_k diag(attn_k)@neigh_k (fp32r, PSUM accum),
           out = catT.T@WaT (fp32r)
  ACT    : exp, attn normalize, PSUM->SBUF copies, tanh
"""

import sys

sys.path.insert(0, "/opt/trn_rl_repo")

import numpy as np
from contextlib import ExitStack

import concourse.bass as bass
import concourse.tile as tile
from concourse import bacc, mybir
from concourse.bass_utils import run_bass_kernel_spmd
from concourse.masks import make_identity

F32 = mybir.dt.float32
F32R = mybir.dt.float32r
U8 = mybir.dt.uint8
ALU = mybir.AluOpType
ACTF = mybir.ActivationFunctionType
AX = mybir.AxisListType

N, K, D = 20000, 32, 512
N_CORES = 8
NS = N // N_CORES  # nodes per core

NEG_BIG = -1.0e30


def build_nc(ns=NS, agg_mode="pe", debug_taps=False):
    """Build the per-core Tile program.

    agg_mode: 'pe'  — aggregate via diag(attn_k) matmuls on the tensor engine
              'dve' — aggregate via scalar_tensor_tensor chain on DVE
    """
    nc = bacc.Bacc("TRN2", target_bir_lowering=False, debug=False)

    prev_d = nc.dram_tensor("prev", [ns, D], F32, kind="ExternalInput").ap()
    # declared f32r so tiles can feed fp32r matmuls; bit-identical to f32
    neigh_d = nc.dram_tensor("neigh", [ns, K, D], F32, kind="ExternalInput").ap()
    mask_d = nc.dram_tensor("mask", [ns, K], U8, kind="ExternalInput").ap()
    w1_d = nc.dram_tensor("W1", [D, D], F32, kind="ExternalInput").ap()
    wa_d = nc.dram_tensor("Wa", [D, 2 * D], F32, kind="ExternalInput").ap()
    out_d = nc.dram_tensor("out", [ns, D], F32, kind="ExternalOutput").ap()
    taps = {}
    if debug_taps:
        for nm, sh in [("x", [ns, D]), ("scores", [ns, K]), ("attn", [ns, K]), ("agg", [ns, D])]:
            taps[nm] = nc.dram_tensor("tap_" + nm, sh, F32, kind="ExternalOutput").ap()

    n_tiles = (ns + 127) // 128
    DC = D // 128  # 4 d-chunks
    CC = 2 * D // 128  # 8 concat-chunks
    NCH = 8  # neigh DMA chunks per tile
    KCH = K // NCH  # k's per chunk

    with tile.TileContext(nc) as tc, ExitStack() as ctx:
        consts = ctx.enter_context(tc.tile_pool(name="consts", bufs=1))
        wstage = ctx.enter_context(tc.tile_pool(name="wstage", bufs=1))
        p_neigh = ctx.enter_context(tc.tile_pool(name="neigh", bufs=2))
        p_io = ctx.enter_context(tc.tile_pool(name="io", bufs=2))
        p_scr = ctx.enter_context(tc.tile_pool(name="scr", bufs=1))
        p_big = ctx.enter_context(tc.tile_pool(name="big", bufs=2))
        p_small = ctx.enter_context(tc.tile_pool(name="small", bufs=2))
        p_diag = ctx.enter_context(tc.tile_pool(name="diag", bufs=1))
        p_ps_tr = ctx.enter_context(tc.tile_pool(name="ps_tr", bufs=2, space="PSUM"))
        p_ps_x = ctx.enter_context(tc.tile_pool(name="ps_x", bufs=2, space="PSUM"))
        p_ps_a = ctx.enter_context(tc.tile_pool(name="ps_a", bufs=2, space="PSUM"))
        p_ps_o = ctx.enter_context(tc.tile_pool(name="ps_o", bufs=2, space="PSUM"))

        # ---- one-time: identity + transposed weights ----
        ident = consts.tile([128, 128], F32)
        make_identity(nc, ident[:])

        # W1T[i, j] = W1[j, i] packed [128, DC, D] (f32: x-matmul is exact fp32)
        w1t = consts.tile([128, DC, D], F32)
        for cj in range(DC):
            wrow = wstage.tile([128, D], F32, tag="wrow")
            nc.sync.dma_start(wrow[:], w1_d[cj * 128 : (cj + 1) * 128, :])
            for ci in range(DC):
                tp = p_ps_tr.tile([128, 128], F32, tag="tp", name="tp")
                nc.tensor.transpose(tp[:], wrow[:, ci * 128 : (ci + 1) * 128], ident[:])
                nc.scalar.copy(w1t[:, ci, cj * 128 : (cj + 1) * 128], tp[:])

        # WaT[c, j] = Wa[j, c] packed [128, CC, D] (f32: out-matmul is exact fp32)
        wat = consts.tile([128, CC, D], F32)
        for cj in range(DC):
            wrow = wstage.tile([128, 2 * D], F32, tag="warow")
            nc.sync.dma_start(wrow[:], wa_d[cj * 128 : (cj + 1) * 128, :])
            for ci in range(CC):
                tp = p_ps_tr.tile([128, 128], F32, tag="tp", name="tp")
                nc.tensor.transpose(tp[:], wrow[:, ci * 128 : (ci + 1) * 128], ident[:])
                nc.scalar.copy(wat[:, ci, cj * 128 : (cj + 1) * 128], tp[:])

        # ---- per-tile loop (software pipelined: x for tile t+1 is computed
        # on PE before tile t's aggregation matmuls so DVE never starves) ----
        state = {}

        def emit_loads(t):
            n0 = t * 128
            P = min(128, ns - n0)
            nghs = []
            for c in range(NCH):
                g = p_neigh.tile([P, KCH, D], F32, tag=f"ng{c}", name=f"ng{c}")
                nc.sync.dma_start(
                    g[:], neigh_d[n0 : n0 + P, KCH * c : KCH * (c + 1), :]
                )
                nghs.append(g)
            prev_t = p_io.tile([P, D], F32, tag="prev", name="prev_t")
            nc.scalar.dma_start(prev_t[:], prev_d[n0 : n0 + P, :])
            mask_t = p_small.tile([P, K], U8, tag="mask", name="mask_t")
            nc.scalar.dma_start(mask_t[:], mask_d[n0 : n0 + P, :])
            return {"nghs": nghs, "prev": prev_t, "mask": mask_t, "P": P, "n0": n0}

        def emit_x(st):
            P = st["P"]
            catT = p_big.tile([128, CC, P], F32, tag="catT", name="catT")
            prevT = p_io.tile([128, DC, P], F32, tag="prevT", name="prevT")
            for ci in range(DC):
                tp = p_ps_tr.tile([128, P], F32, tag="tp", name="tp")
                nc.tensor.transpose(
                    tp[:], st["prev"][:, ci * 128 : (ci + 1) * 128], ident[:P, :P]
                )
                nc.scalar.copy(prevT[:, ci, :], tp[:])
                nc.scalar.copy(catT[:, DC + ci, :], tp[:])
            x_ps = p_ps_x.tile([P, D], F32, tag="x", name="x_ps")
            for ci in range(DC):
                nc.tensor.matmul(
                    x_ps[:],
                    prevT[:, ci, :],
                    w1t[:, ci, :],
                    start=(ci == 0),
                    stop=(ci == DC - 1),
                )
            x_sb = p_io.tile([P, D], F32, tag="x_sb", name="x_sb")
            nc.scalar.copy(x_sb[:], x_ps[:])
            if debug_taps:
                nc.sync.dma_start(taps["x"][st["n0"] : st["n0"] + P, :], x_sb[:])
            st["catT"] = catT
            st["x_sb"] = x_sb

        def emit_body(st):
            P, n0 = st["P"], st["n0"]
            nghs, x_sb, catT = st["nghs"], st["x_sb"], st["catT"]

            def ng(k):
                return nghs[k // KCH][:, k % KCH, :]

            # mask penalty (u8 -> f32 * -1e30)
            maskpen = p_small.tile([P, K], F32, tag="maskpen", name="maskpen")
            nc.vector.tensor_scalar_mul(maskpen[:], st["mask"][:], NEG_BIG)

            # scores_k = sum_d neigh_k * x  (fused dot via scalar_tensor_tensor
            # accum_out; InstTensorTensorReduce crashes TRN2 hw here)
            scores = p_small.tile([P, K], F32, tag="scores", name="scores")
            scratch = p_scr.tile([P, D], F32, tag="scratch", name="scratch")
            for k in range(K):
                nc.vector.scalar_tensor_tensor(
                    out=scratch[:],
                    in0=ng(k),
                    scalar=1.0,
                    in1=x_sb[:],
                    op0=ALU.bypass,
                    op1=ALU.mult,
                    accum_out=scores[:, k : k + 1],
                )
            nc.vector.tensor_tensor(
                out=scores[:], in0=scores[:], in1=maskpen[:], op=ALU.add
            )

            if debug_taps:
                nc.sync.dma_start(taps["scores"][n0 : n0 + P, :], scores[:])
            # softmax over k (free dim)
            nmx = p_small.tile([P, 1], F32, tag="nmx", name="nmx")
            nc.vector.tensor_reduce(
                nmx[:], scores[:], axis=AX.X, op=ALU.max, negate=True
            )
            ex = p_small.tile([P, K], F32, tag="ex", name="ex")
            nc.scalar.activation(ex[:], scores[:], ACTF.Exp, bias=nmx[:, 0:1], scale=1.0)
            ssum = p_small.tile([P, 1], F32, tag="ssum", name="ssum")
            nc.vector.tensor_reduce(ssum[:], ex[:], axis=AX.X, op=ALU.add)
            rec = p_small.tile([P, 1], F32, tag="rec", name="rec")
            nc.vector.reciprocal(rec[:], ssum[:])
            attn = p_small.tile([P, K], F32, tag="attn", name="attn")
            nc.scalar.activation(attn[:], ex[:], ACTF.Copy, bias=0.0, scale=rec[:, 0:1])

            if debug_taps:
                nc.sync.dma_start(taps["attn"][n0 : n0 + P, :], attn[:])
            if agg_mode == "pe":
                # split aggregation: k < NPE via plain-fp32 diag(attn_k)
                # matmuls on PE (exact), the rest on DVE; engines balanced
                NPE = 20
                diags = p_diag.tile([128, NPE, 128], F32, tag="diags", name="diags")
                attn_b = (
                    attn[:, :NPE].unsqueeze(2).broadcast_to([P, NPE, 128])
                )
                nc.gpsimd.affine_select(
                    out=diags[:P, :, :128],
                    in_=attn_b,
                    pattern=[[0, NPE], [-1, 128]],
                    compare_op=ALU.is_equal,
                    fill=0.0,
                    base=0,
                    channel_multiplier=1,
                )
                agg_ps = p_ps_a.tile([P, D], F32, tag="agg", name="agg_ps")
                for k in range(NPE):
                    nc.tensor.matmul(
                        agg_ps[:],
                        diags[:P, k, :P],
                        ng(k),
                        start=(k == 0),
                        stop=(k == NPE - 1),
                    )
                agg = p_scr.tile([P, D], F32, tag="agg", name="agg")
                nc.vector.scalar_tensor_tensor(
                    out=agg[:], in0=ng(NPE), scalar=attn[:, NPE : NPE + 1],
                    in1=x_sb[:], op0=ALU.mult, op1=ALU.bypass,
                )
                for k in range(NPE + 1, K):
                    nc.vector.scalar_tensor_tensor(
                        out=agg[:], in0=ng(k), scalar=attn[:, k : k + 1],
                        in1=agg[:], op0=ALU.mult, op1=ALU.add,
                    )
                nc.vector.tensor_tensor(
                    out=agg[:], in0=agg[:], in1=agg_ps[:], op=ALU.add
                )
            else:
                agg = p_io.tile([P, D], F32, tag="agg", name="agg")
                nc.vector.scalar_tensor_tensor(
                    out=agg[:], in0=ng(0), scalar=attn[:, 0:1],
                    in1=x_sb[:], op0=ALU.mult, op1=ALU.bypass,
                )
                for k in range(1, K):
                    nc.vector.scalar_tensor_tensor(
                        out=agg[:], in0=ng(k), scalar=attn[:, k : k + 1],
                        in1=agg[:], op0=ALU.mult, op1=ALU.add,
                    )

            # aggT into catT chunks 0..DC-1
            for ci in range(DC):
                tp = p_ps_tr.tile([128, P], F32, tag="tp", name="tp")
                nc.tensor.transpose(
                    tp[:], agg[:, ci * 128 : (ci + 1) * 128], ident[:P, :P]
                )
                nc.scalar.copy(catT[:, ci, :], tp[:])

            # out = tanh(cat @ Wa.T)   (plain fp32: 1024-long contraction is
            # too error-prone in fp32r)
            o_ps = p_ps_o.tile([P, D], F32, tag="o", name="o_ps")
            for ci in range(CC):
                nc.tensor.matmul(
                    o_ps[:],
                    catT[:, ci, :],
                    wat[:, ci, :],
                    start=(ci == 0),
                    stop=(ci == CC - 1),
                )
            out_sb = p_io.tile([P, D], F32, tag="out_sb", name="out_sb")
            nc.scalar.activation(out_sb[:], o_ps[:], ACTF.Tanh)
            nc.scalar.dma_start(out_d[n0 : n0 + P, :], out_sb[:])

        # prologue
        cur = emit_loads(0)
        emit_x(cur)
        for t in range(n_tiles):
            nxt = None
            if t + 1 < n_tiles:
                nxt = emit_loads(t + 1)
                emit_x(nxt)
            emit_body(cur)
            cur = nxt

    nc.compile()
    return nc


_NC_CACHE = {}


def _get_nc(ns=NS, agg_mode="pe"):
    key = (ns, agg_mode)
    if key not in _NC_CACHE:
        _NC_CACHE[key] = build_nc(ns, agg_mode)
    return _NC_CACHE[key]


def kernel(prev_hidden, neigh_hidden, mask, W1, Wa, _trace=False, _tmpdir=None):
    prev_hidden = np.ascontiguousarray(prev_hidden, dtype=np.float32)
    neigh_hidden = np.ascontiguousarray(neigh_hidden, dtype=np.float32)
    mask_u8 = np.ascontiguousarray(mask).view(np.uint8)
    W1 = np.ascontiguousarray(W1, dtype=np.float32)
    Wa = np.ascontiguousarray(Wa, dtype=np.float32)

    nc = _get_nc()
    in_maps = []
    for c in range(N_CORES):
        s = slice(c * NS, (c + 1) * NS)
        in_maps.append(
            {
                "prev": prev_hidden[s],
                "neigh": neigh_hidden[s],
                "mask": mask_u8[s],
                "W1": W1,
                "Wa": Wa,
            }
        )
    res = run_bass_kernel_spmd(
        nc,
        in_maps,
        core_ids=list(range(N_CORES)),
        trace=_trace,
        tmpdir=_tmpdir,
    )
    out = np.concatenate([res.results[c]["out"] for c in range(N_CORES)], axis=0)
    if _trace:
        kernel.last_exec_time_ns = res.exec_time_ns
    return out


# revision 15
# speedup vs baseline: 1.5251x; 1.1284x over previous
"""Trainium2 Bass kernel for AttentionAggregator (GNN message passing).

  x      = prev_hidden @ W1.T                       [N, D]
  scores = einsum('nkd,nd->nk', neigh_hidden, x)    [N, K]
  attn   = softmax(where(mask, -inf, scores), k)    [N, K]
  agg    = einsum('nk,nkd->nd', attn, neigh_hidden) [N, K, D] -> [N, D]
  out    = tanh(concat([agg, prev_hidden], 1) @ Wa.T)

Sharding: node dim N split evenly across 8 NeuronCores (data parallel);
W1 / Wa replicated.  Per-core kernel is DMA-bound (neigh_hidden stream,
~164MB/core) with DVE as the #2 engine.

Per-tile schedule (P<=128 nodes on partitions):
  DVE    : scores via fused dot (scalar_tensor_tensor accum_out), softmax
           reductions, reciprocal
  GPSIMD : builds diag(attn[:,k]) for all k in one affine_select
  PE     : prev/agg transposes, x = prevT.T@W1T (plain fp32 for exact
           logits), agg = sum_k diag(attn_k)@neigh_k (fp32r, PSUM accum),
           out = catT.T@WaT (fp32r)
  ACT    : exp, attn normalize, PSUM->SBUF copies, tanh
"""

import sys

sys.path.insert(0, "/opt/trn_rl_repo")

import numpy as np
from contextlib import ExitStack

import concourse.bass as bass
import concourse.tile as tile
from concourse import bacc, mybir
from concourse.bass_utils import run_bass_kernel_spmd
from concourse.masks import make_identity

F32 = mybir.dt.float32
F32R = mybir.dt.float32r
U8 = mybir.dt.uint8
ALU = mybir.AluOpType
ACTF = mybir.ActivationFunctionType
AX = mybir.AxisListType

N, K, D = 20000, 32, 512
N_CORES = 8
NS = N // N_CORES  # nodes per core

NEG_BIG = -1.0e30


def build_nc(ns=NS, agg_mode="pe2", debug_taps=False):
    """Build the per-core Tile program.

    agg_mode: 'pe'  — aggregate via diag(attn_k) matmuls on the tensor engine
              'dve' — aggregate via scalar_tensor_tensor chain on DVE
    """
    nc = bacc.Bacc("TRN2", target_bir_lowering=False, debug=False)

    prev_d = nc.dram_tensor("prev", [ns, D], F32, kind="ExternalInput").ap()
    # declared f32r so tiles can feed fp32r matmuls; bit-identical to f32
    neigh_d = nc.dram_tensor("neigh", [ns, K, D], F32, kind="ExternalInput").ap()
    mask_d = nc.dram_tensor("mask", [ns, K], U8, kind="ExternalInput").ap()
    w1_d = nc.dram_tensor("W1", [D, D], F32, kind="ExternalInput").ap()
    wa_d = nc.dram_tensor("Wa", [D, 2 * D], F32, kind="ExternalInput").ap()
    out_d = nc.dram_tensor("out", [ns, D], F32, kind="ExternalOutput").ap()
    taps = {}
    if debug_taps:
        for nm, sh in [("x", [ns, D]), ("scores", [ns, K]), ("attn", [ns, K]), ("agg", [ns, D])]:
            taps[nm] = nc.dram_tensor("tap_" + nm, sh, F32, kind="ExternalOutput").ap()

    n_tiles = (ns + 127) // 128
    DC = D // 128  # 4 d-chunks
    CC = 2 * D // 128  # 8 concat-chunks
    NCH = 8  # neigh DMA chunks per tile
    KCH = K // NCH  # k's per chunk

    with tile.TileContext(nc) as tc, ExitStack() as ctx:
        consts = ctx.enter_context(tc.tile_pool(name="consts", bufs=1))
        wstage = ctx.enter_context(tc.tile_pool(name="wstage", bufs=1))
        p_neigh = ctx.enter_context(tc.tile_pool(name="neigh", bufs=2))
        p_io = ctx.enter_context(tc.tile_pool(name="io", bufs=2))
        p_scr = ctx.enter_context(tc.tile_pool(name="scr", bufs=1))
        p_big = ctx.enter_context(tc.tile_pool(name="big", bufs=2))
        p_small = ctx.enter_context(tc.tile_pool(name="small", bufs=2))
        p_diag = ctx.enter_context(tc.tile_pool(name="diag", bufs=1))
        p_stg = ctx.enter_context(tc.tile_pool(name="stg", bufs=3))
        p_ps_tr = ctx.enter_context(tc.tile_pool(name="ps_tr", bufs=2, space="PSUM"))
        p_ps_x = ctx.enter_context(tc.tile_pool(name="ps_x", bufs=2, space="PSUM"))
        p_ps_a = ctx.enter_context(tc.tile_pool(name="ps_a", bufs=2, space="PSUM"))
        p_ps_o = ctx.enter_context(tc.tile_pool(name="ps_o", bufs=2, space="PSUM"))

        # ---- one-time: identity + transposed weights ----
        ident = consts.tile([128, 128], F32)
        make_identity(nc, ident[:])

        # W1T[i, j] = W1[j, i] packed [128, DC, D] (f32: x-matmul is exact fp32)
        w1t = consts.tile([128, DC, D], F32)
        for cj in range(DC):
            wrow = wstage.tile([128, D], F32, tag="wrow")
            nc.sync.dma_start(wrow[:], w1_d[cj * 128 : (cj + 1) * 128, :])
            for ci in range(DC):
                tp = p_ps_tr.tile([128, 128], F32, tag="tp", name="tp")
                nc.tensor.transpose(tp[:], wrow[:, ci * 128 : (ci + 1) * 128], ident[:])
                nc.scalar.copy(w1t[:, ci, cj * 128 : (cj + 1) * 128], tp[:])

        # WaT[c, j] = Wa[j, c] packed [128, CC, D] (f32: out-matmul is exact fp32)
        wat = consts.tile([128, CC, D], F32)
        for cj in range(DC):
            wrow = wstage.tile([128, 2 * D], F32, tag="warow")
            nc.sync.dma_start(wrow[:], wa_d[cj * 128 : (cj + 1) * 128, :])
            for ci in range(CC):
                tp = p_ps_tr.tile([128, 128], F32, tag="tp", name="tp")
                nc.tensor.transpose(tp[:], wrow[:, ci * 128 : (ci + 1) * 128], ident[:])
                nc.scalar.copy(wat[:, ci, cj * 128 : (cj + 1) * 128], tp[:])

        # ---- per-tile loop (software pipelined: x for tile t+1 is computed
        # on PE before tile t's aggregation matmuls so DVE never starves) ----
        state = {}

        def emit_loads(t):
            n0 = t * 128
            P = min(128, ns - n0)
            nghs = []
            for c in range(NCH):
                g = p_neigh.tile([P, KCH, D], F32, tag=f"ng{c}", name=f"ng{c}")
                nc.sync.dma_start(
                    g[:], neigh_d[n0 : n0 + P, KCH * c : KCH * (c + 1), :]
                )
                nghs.append(g)
            prev_t = p_io.tile([P, D], F32, tag="prev", name="prev_t")
            nc.scalar.dma_start(prev_t[:], prev_d[n0 : n0 + P, :])
            mask_t = p_small.tile([P, K], U8, tag="mask", name="mask_t")
            nc.scalar.dma_start(mask_t[:], mask_d[n0 : n0 + P, :])
            return {"nghs": nghs, "prev": prev_t, "mask": mask_t, "P": P, "n0": n0}

        def emit_x(st):
            P = st["P"]
            catT = p_big.tile([128, CC, P], F32, tag="catT", name="catT")
            prevT = p_io.tile([128, DC, P], F32, tag="prevT", name="prevT")
            for ci in range(DC):
                tp = p_ps_tr.tile([128, P], F32, tag="tp", name="tp")
                nc.tensor.transpose(
                    tp[:], st["prev"][:, ci * 128 : (ci + 1) * 128], ident[:P, :P]
                )
                nc.scalar.copy(prevT[:, ci, :], tp[:])
                nc.scalar.copy(catT[:, DC + ci, :], tp[:])
            x_ps = p_ps_x.tile([P, D], F32, tag="x", name="x_ps")
            for ci in range(DC):
                nc.tensor.matmul(
                    x_ps[:],
                    prevT[:, ci, :],
                    w1t[:, ci, :],
                    start=(ci == 0),
                    stop=(ci == DC - 1),
                )
            x_sb = p_io.tile([P, D], F32, tag="x_sb", name="x_sb")
            nc.scalar.copy(x_sb[:], x_ps[:])
            if debug_taps:
                nc.sync.dma_start(taps["x"][st["n0"] : st["n0"] + P, :], x_sb[:])
            st["catT"] = catT
            st["x_sb"] = x_sb

        def emit_body(st):
            P, n0 = st["P"], st["n0"]
            nghs, x_sb, catT = st["nghs"], st["x_sb"], st["catT"]

            def ng(k):
                return nghs[k // KCH][:, k % KCH, :]

            # mask penalty (u8 -> f32 * -1e30)
            maskpen = p_small.tile([P, K], F32, tag="maskpen", name="maskpen")
            nc.vector.tensor_scalar_mul(maskpen[:], st["mask"][:], NEG_BIG)

            # scores_k = sum_d neigh_k * x  (fused dot via scalar_tensor_tensor
            # accum_out; InstTensorTensorReduce crashes TRN2 hw here)
            scores = p_small.tile([P, K], F32, tag="scores", name="scores")
            scratch = p_scr.tile([P, D], F32, tag="scratch", name="scratch")
            for k in range(K):
                nc.vector.scalar_tensor_tensor(
                    out=scratch[:],
                    in0=ng(k),
                    scalar=1.0,
                    in1=x_sb[:],
                    op0=ALU.bypass,
                    op1=ALU.mult,
                    accum_out=scores[:, k : k + 1],
                )
            nc.vector.tensor_tensor(
                out=scores[:], in0=scores[:], in1=maskpen[:], op=ALU.add
            )

            if debug_taps:
                nc.sync.dma_start(taps["scores"][n0 : n0 + P, :], scores[:])
            # softmax over k (free dim)
            nmx = p_small.tile([P, 1], F32, tag="nmx", name="nmx")
            nc.vector.tensor_reduce(
                nmx[:], scores[:], axis=AX.X, op=ALU.max, negate=True
            )
            ex = p_small.tile([P, K], F32, tag="ex", name="ex")
            nc.scalar.activation(ex[:], scores[:], ACTF.Exp, bias=nmx[:, 0:1], scale=1.0)
            ssum = p_small.tile([P, 1], F32, tag="ssum", name="ssum")
            nc.vector.tensor_reduce(ssum[:], ex[:], axis=AX.X, op=ALU.add)
            rec = p_small.tile([P, 1], F32, tag="rec", name="rec")
            nc.vector.reciprocal(rec[:], ssum[:])
            attn = p_small.tile([P, K], F32, tag="attn", name="attn")
            nc.scalar.activation(attn[:], ex[:], ACTF.Copy, bias=0.0, scale=rec[:, 0:1])

            if debug_taps:
                nc.sync.dma_start(taps["attn"][n0 : n0 + P, :], attn[:])
            if agg_mode == "pe2":
                # f32r staging copies of neigh on ACT (the idle engine);
                # agg entirely on PE via f32r diag matmuls
                stgs = []
                for c in range(K // 2):
                    stg = p_stg.tile([P, 2, D], F32R, tag="stg", name="stg")
                    src = nghs[(2 * c) // KCH][:, (2 * c) % KCH : (2 * c) % KCH + 2, :]
                    nc.scalar.copy(stg[:], src)
                    stgs.append(stg)

                def ngr(k):
                    return stgs[k // 2][:, k % 2, :]

                KH = K // 2
                agg_ps = p_ps_a.tile([P, D], F32, tag="agg", name="agg_ps")
                for h in range(2):
                    diags = p_diag.tile([128, KH, 128], F32R, tag="diags", name="diags")
                    attn_b = (
                        attn[:, h * KH : (h + 1) * KH]
                        .unsqueeze(2)
                        .broadcast_to([P, KH, 128])
                    )
                    nc.gpsimd.affine_select(
                        out=diags[:P, :, :128],
                        in_=attn_b,
                        pattern=[[0, KH], [-1, 128]],
                        compare_op=ALU.is_equal,
                        fill=0.0,
                        base=0,
                        channel_multiplier=1,
                    )
                    for kk in range(KH):
                        k = h * KH + kk
                        nc.tensor.matmul(
                            agg_ps[:],
                            diags[:P, kk, :P],
                            ngr(k),
                            start=(k == 0),
                            stop=(k == K - 1),
                        )
                agg = p_scr.tile([P, D], F32, tag="agg", name="agg")
                nc.scalar.copy(agg[:], agg_ps[:])
            elif agg_mode == "pe":
                # split aggregation: k < NPE via plain-fp32 diag(attn_k)
                # matmuls on PE (exact), the rest on DVE; engines balanced
                NPE = 20
                diags = p_diag.tile([128, NPE, 128], F32, tag="diags", name="diags")
                attn_b = (
                    attn[:, :NPE].unsqueeze(2).broadcast_to([P, NPE, 128])
                )
                nc.gpsimd.affine_select(
                    out=diags[:P, :, :128],
                    in_=attn_b,
                    pattern=[[0, NPE], [-1, 128]],
                    compare_op=ALU.is_equal,
                    fill=0.0,
                    base=0,
                    channel_multiplier=1,
                )
                agg_ps = p_ps_a.tile([P, D], F32, tag="agg", name="agg_ps")
                for k in range(NPE):
                    nc.tensor.matmul(
                        agg_ps[:],
                        diags[:P, k, :P],
                        ng(k),
                        start=(k == 0),
                        stop=(k == NPE - 1),
                    )
                agg = p_scr.tile([P, D], F32, tag="agg", name="agg")
                nc.vector.scalar_tensor_tensor(
                    out=agg[:], in0=ng(NPE), scalar=attn[:, NPE : NPE + 1],
                    in1=x_sb[:], op0=ALU.mult, op1=ALU.bypass,
                )
                for k in range(NPE + 1, K):
                    nc.vector.scalar_tensor_tensor(
                        out=agg[:], in0=ng(k), scalar=attn[:, k : k + 1],
                        in1=agg[:], op0=ALU.mult, op1=ALU.add,
                    )
                nc.vector.tensor_tensor(
                    out=agg[:], in0=agg[:], in1=agg_ps[:], op=ALU.add
                )
            else:
                agg = p_io.tile([P, D], F32, tag="agg", name="agg")
                nc.vector.scalar_tensor_tensor(
                    out=agg[:], in0=ng(0), scalar=attn[:, 0:1],
                    in1=x_sb[:], op0=ALU.mult, op1=ALU.bypass,
                )
                for k in range(1, K):
                    nc.vector.scalar_tensor_tensor(
                        out=agg[:], in0=ng(k), scalar=attn[:, k : k + 1],
                        in1=agg[:], op0=ALU.mult, op1=ALU.add,
                    )

            # aggT into catT chunks 0..DC-1
            for ci in range(DC):
                tp = p_ps_tr.tile([128, P], F32, tag="tp", name="tp")
                nc.tensor.transpose(
                    tp[:], agg[:, ci * 128 : (ci + 1) * 128], ident[:P, :P]
                )
                nc.scalar.copy(catT[:, ci, :], tp[:])

            # out = tanh(cat @ Wa.T)   (plain fp32: 1024-long contraction is
            # too error-prone in fp32r)
            o_ps = p_ps_o.tile([P, D], F32, tag="o", name="o_ps")
            for ci in range(CC):
                nc.tensor.matmul(
                    o_ps[:],
                    catT[:, ci, :],
                    wat[:, ci, :],
                    start=(ci == 0),
                    stop=(ci == CC - 1),
                )
            out_sb = p_io.tile([P, D], F32, tag="out_sb", name="out_sb")
            nc.scalar.activation(out_sb[:], o_ps[:], ACTF.Tanh)
            nc.scalar.dma_start(out_d[n0 : n0 + P, :], out_sb[:])

        # prologue
        cur = emit_loads(0)
        emit_x(cur)
        for t in range(n_tiles):
            nxt = None
            if t + 1 < n_tiles:
                nxt = emit_loads(t + 1)
                emit_x(nxt)
            emit_body(cur)
            cur = nxt

    nc.compile()
    return nc


_NC_CACHE = {}


def _get_nc(ns=NS, agg_mode="pe2"):
    key = (ns, agg_mode)
    if key not in _NC_CACHE:
        _NC_CACHE[key] = build_nc(ns, agg_mode)
    return _NC_CACHE[key]


def kernel(prev_hidden, neigh_hidden, mask, W1, Wa, _trace=False, _tmpdir=None):
    prev_hidden = np.ascontiguousarray(prev_hidden, dtype=np.float32)
    neigh_hidden = np.ascontiguousarray(neigh_hidden, dtype=np.float32)
    mask_u8 = np.ascontiguousarray(mask).view(np.uint8)
    W1 = np.ascontiguousarray(W1, dtype=np.float32)
    Wa = np.ascontiguousarray(Wa, dtype=np.float32)

    nc = _get_nc()
    in_maps = []
    for c in range(N_CORES):
        s = slice(c * NS, (c + 1) * NS)
        in_maps.append(
            {
                "prev": prev_hidden[s],
                "neigh": neigh_hidden[s],
                "mask": mask_u8[s],
                "W1": W1,
                "Wa": Wa,
            }
        )
    res = run_bass_kernel_spmd(
        nc,
        in_maps,
        core_ids=list(range(N_CORES)),
        trace=_trace,
        tmpdir=_tmpdir,
    )
    out = np.concatenate([res.results[c]["out"] for c in range(N_CORES)], axis=0)
    if _trace:
        kernel.last_exec_time_ns = res.exec_time_ns
    return out
